# revision 10
# baseline (speedup 1.0000x reference)
"""Multi-head attention (B=2, C=64, H=W=64, nh=8) on 8 TRN2 NeuronCores.

Sharding: core = (batch b, head-pair hp): b = core//4, heads {2hp, 2hp+1}
with hp = core%4.  Attention is independent per (batch, head), so 16 units
spread over 8 cores with zero communication; conv1x1 weights are sliced
per-core (each core only computes the 48 q/k/v output channels its two
heads need).

Per-core device pipeline (scores never hit HBM):
  x[b] bf16 [64,4096] -> conv1x1 on PE (bias via ones row) -> A [48,4096]
  bf16 (rows: q 2x8 | k 2x8 | v 2x8, torch-.view-scrambled layout
  A[row, 8t+dd]).
  - DVE deinterleave: Bqk [32, (dd,t)] bf16 (strided in-partition copy;
    kills the 2-byte-granular DMA gathers that dominated the first
    kernel -- Q^T/K^T gathers below move 1KB contiguous runs).
  - DRAM bounce: Bqk -> scrB, A_v -> scrA; per head gather
    qt/kt [8,4096] bf16 and vc [128,32,33] bf16 (16-byte runs; col 32 =
    ones for the softmax denominator).  All gathers for both heads are
    issued before the attention loops so head 1's DMA overlaps head 0's
    compute.
  - S^T tiles [128m, 512n] = K Q^T on PE (bf16), exp on ACT with
    1/sqrt(8) folded into the activation scale (scores are O(1) by
    construction; no max subtraction), PV + denominator via augmented
    V|ones bf16 matmul, normalize with PE ones-broadcast + DVE
    reciprocal.  The PV accumulator psum is copied to SBUF immediately so
    the slow reciprocal sits off the PV-accumulate critical path.
  Device returns gamma*attn^T per head ([dd, n] layout, bf16); the host
  un-views to [B,C,H,W] and adds the residual x in f32 (gamma is folded
  into the V weights).

Measured on hw: PE issues one 512-col matmul per ~427ns regardless of
dtype (fp8 DoubleRow only deepens contraction, useless at hd=8), so the
1024 S+PV matmuls put the per-core floor at ~440us; ACT exp is ~264us.
"""

import os

os.environ.setdefault("NEURON_RT_RESET_CORES", "1")  # recover wedged cores

import numpy as np
import ml_dtypes

import concourse.bacc as bacc
import concourse.bass as bass
import concourse.tile as tile
from concourse import mybir
from concourse.bass_utils import run_bass_kernel_spmd

F32 = mybir.dt.float32
F32R = mybir.dt.float32r
BF16 = mybir.dt.bfloat16

B = 2
C = 64
N = 4096          # H*W
NH = 8
HD = 8            # head dim
NCORES = 8
HPC = 2           # heads per core
NBLK = N // 512   # 8 n-blocks of 512 query positions
MCHUNK = N // 128  # 32 m-chunks of 128 key positions
EXPW = 1536       # elements exp'd per ACT instruction (psum banks = EXPW/512)
SCALE = 1.0 / np.sqrt(float(HD))
NPBF16 = ml_dtypes.bfloat16
NROWS = 3 * HPC * HD  # 48 conv output channels per core


def _chunk_groups():
    """Partition the 32 m-chunks into groups of <= EXPW//512 for one exp each."""
    per = EXPW // 512
    groups, k = [], 0
    while k < MCHUNK:
        n = min(per, MCHUNK - k)
        groups.append(list(range(k, k + n)))
        k += n
    return groups


def _emit(tc, xb_d, wcat_d, out_d, scrA, scrB):
    nc = tc.nc

    with (
        tc.tile_pool(name="persist", bufs=1) as per,
        tc.tile_pool(name="ptp", bufs=3) as ptp,
        tc.tile_pool(name="epl", bufs=4) as epl,
        tc.tile_pool(name="hdp", bufs=2) as hdp,
        tc.tile_pool(name="stp", bufs=2, space="PSUM") as stp,
        tc.tile_pool(name="accp", bufs=2, space="PSUM") as accp,
    ):
        # ---- persistent: fp32r ones row for the denominator broadcast ----
        ones8r = per.tile([1, HD], F32R)
        o8f = per.tile([1, HD], F32)
        nc.vector.memset(o8f, 1.0)
        nc.vector.tensor_copy(ones8r, o8f)  # rounds to fp32r

        # ---- conv1x1: [48,4096] = wcat.T @ [65,4096] ----
        with tc.tile_pool(name="convin", bufs=1) as cin:
            xa = cin.tile([C + 1, N], BF16)  # x[b] + ones row (bias)
            nc.vector.memset(xa[C : C + 1, :], 1.0)
            nc.sync.dma_start(out=xa[0:C, :], in_=xb_d[:])
            wc = cin.tile([C + 1, NROWS], BF16)
            nc.sync.dma_start(out=wc[:], in_=wcat_d[:])

            av = cin.tile([NROWS, N], BF16, name="av")  # conv out
            for j in range(NBLK):
                ps = stp.tile([128, EXPW], F32, tag="st")
                nc.tensor.matmul(
                    ps[0:NROWS, 0:512],
                    lhsT=wc[:],
                    rhs=xa[:, j * 512 : (j + 1) * 512],
                    start=True,
                    stop=True,
                )
                nc.vector.tensor_copy(
                    av[:, j * 512 : (j + 1) * 512], ps[0:NROWS, 0:512]
                )
            # V rows (natural layout) bounce to DRAM for the 16B-run gather
            nc.sync.dma_start(out=scrA[:], in_=av[32:48, :])
            # Q,K rows: deinterleave head-dim on DVE, bounce for 1KB gathers
            bqk = cin.tile([32, N], BF16, name="bqk")
            nc.vector.tensor_copy(
                bqk[:].rearrange("p (d t) -> p d t", d=HD),
                av[0:32, :].rearrange("p (t d) -> p d t", d=HD),
            )
            nc.sync.dma_start(out=scrB[:], in_=bqk[:])

        # ---- gathers for BOTH heads up front (overlap head 0 compute) ----
        qt, kt, vc = {}, {}, {}
        for h in range(HPC):
            qt[h] = hdp.tile([HD, N], BF16, name=f"qt{h}", tag="qt")
            kt[h] = hdp.tile([HD, N], BF16, name=f"kt{h}", tag="kt")
            vc[h] = hdp.tile([128, MCHUNK, 33], BF16, name=f"vc{h}", tag="vc")

            # qt[dd, 512r+t] = scrB[8h+r, dd*512 + t]  (1KB runs)
            nc.sync.dma_start(
                out=qt[h][:].rearrange("d (r t) -> d r t", r=HD),
                in_=scrB[h * HD : (h + 1) * HD, :].rearrange(
                    "r (d t) -> d r t", d=HD
                ),
            )
            nc.sync.dma_start(
                out=kt[h][:].rearrange("d (r t) -> d r t", r=HD),
                in_=scrB[16 + h * HD : 16 + (h + 1) * HD, :].rearrange(
                    "r (d t) -> d r t", d=HD
                ),
            )
            # V chunked [i, chunk, d]: m = 128*chunk + i, chunk = 4r+tb;
            # split into row-pair DMAs so the gather spreads across queues
            # and PV's first chunks are ready sooner.
            for q in range(4):
                rows = slice(h * HD + 2 * q, h * HD + 2 * q + 2)
                nc.sync.dma_start(
                    out=vc[h][:, 8 * q : 8 * q + 8, 0:HD],
                    in_=scrA[rows, :].rearrange(
                        "r (tb i d) -> i (r tb) d", tb=4, i=128, d=HD
                    ),
                )
            nc.vector.memset(vc[h][:, :, HD:32], 0.0)
            nc.vector.memset(vc[h][:, :, 32:33], 1.0)

        # ---- attention per (head, n-block) ----
        # Two-level software pipeline on the in-order PE queue:
        #  * 2-group lookahead: groups g+1, g+2's S-matmuls are emitted
        #    before group g's PV-matmuls so PE never waits on ACT's exp
        #    (or its cross-engine semaphore latency).
        #  * deferred epilogue: block j's denominator-broadcast matmul is
        #    emitted only after block j+1's matmuls, so PE doesn't sit
        #    behind the DVE denominator copy it depends on.
        groups = _chunk_groups()
        pending_epi = None
        for h in range(HPC):
            for j in range(NBLK):
                qblk = qt[h][:, j * 512 : (j + 1) * 512]
                acc = accp.tile([33, 512], F32, tag="acc")

                def emit_s(gi, h=h, qblk=qblk):
                    st = stp.tile([128, EXPW], F32, tag="st")
                    for u, k in enumerate(groups[gi]):
                        nc.tensor.matmul(
                            st[:, u * 512 : (u + 1) * 512],
                            lhsT=kt[h][:, k * 128 : (k + 1) * 128],
                            rhs=qblk,
                            start=True,
                            stop=True,
                        )
                    return st

                sts = [emit_s(0)]
                for gi, grp in enumerate(groups):
                    w = len(grp) * 512
                    st_cur = sts.pop(0)
                    pt = ptp.tile([128, EXPW], BF16)
                    w0 = min(1024, w)
                    nc.scalar.activation(
                        pt[:, 0:w0], st_cur[:, 0:w0],
                        mybir.ActivationFunctionType.Exp, scale=SCALE
                    )
                    if w > w0:
                        nc.scalar.activation(
                            pt[:, w0:w], st_cur[:, w0:w],
                            mybir.ActivationFunctionType.Exp, scale=SCALE
                        )
                    if gi + 1 < len(groups):
                        sts.append(emit_s(gi + 1))
                    if gi == 1 and pending_epi is not None:
                        pending_epi()
                        pending_epi = None
                    for u, k in enumerate(grp):
                        nc.tensor.matmul(
                            acc[:, :],
                            lhsT=vc[h][:, k, :],
                            rhs=pt[:, u * 512 : (u + 1) * 512],
                            start=(k == 0),
                            stop=(k == MCHUNK - 1),
                        )

                # ---- epilogue: out = pv / denom (gamma pre-folded in V) ----
                # Copy psum acc to SBUF right away (frees acc for block j+2's
                # PV); the broadcast matmul + reciprocal are DEFERRED into
                # block j+1's emission so they never stall the PE queue.
                pv = epl.tile([HD, 512], F32, tag="pv")
                nc.vector.tensor_copy(pv, acc[0:HD, :])
                sb = epl.tile([1, 512], F32R, tag="sb")
                nc.vector.tensor_copy(sb, acc[32:33, :])  # denom -> fp32r

                def epilogue(h=h, j=j, pv=pv, sb=sb):
                    rbt = stp.tile([128, EXPW], F32, name="rbt", tag="st")
                    rb = rbt[0:HD, 0:512]
                    nc.tensor.matmul(
                        rb, lhsT=ones8r, rhs=sb, start=True, stop=True
                    )
                    rbs = epl.tile([HD, 512], F32, tag="rbs")
                    nc.vector.reciprocal(rbs, rb)
                    fin = epl.tile([HD, 512], BF16, tag="fin")
                    nc.vector.tensor_mul(fin, pv, rbs)
                    nc.gpsimd.dma_start(
                        out=out_d[
                            h * HD : (h + 1) * HD, j * 512 : (j + 1) * 512
                        ],
                        in_=fin,
                    )

                pending_epi = epilogue
        pending_epi()


def build_bass():
    nc = bacc.Bacc("TRN2", target_bir_lowering=False, debug=False, num_devices=NCORES)
    xb_d = nc.dram_tensor("xb", [C, N], BF16, kind="ExternalInput").ap()
    wcat_d = nc.dram_tensor("wcat", [C + 1, NROWS], BF16, kind="ExternalInput").ap()
    out_d = nc.dram_tensor("out", [HPC * HD, N], BF16, kind="ExternalOutput").ap()
    scrA = nc.dram_tensor("scrA", [16, N], BF16).ap()
    scrB = nc.dram_tensor("scrB", [32, N], BF16).ap()

    with tile.TileContext(nc) as tc:
        _emit(tc, xb_d, wcat_d, out_d, scrA, scrB)
    nc.finalize()
    return nc


_NC = None


def _get_nc():
    global _NC
    if _NC is None:
        _NC = build_bass()
    return _NC


def make_in_maps(x, wq, bq, wk, bk, wv, bv, gamma):
    x = np.asarray(x, np.float32)
    g = float(np.asarray(gamma, np.float32).reshape(-1)[0])
    in_maps = []
    for core in range(NCORES):
        b, hp = divmod(core, 4)
        wcat = np.empty((C + 1, NROWS), np.float32)
        for sec, (w, bias, s) in enumerate(
            ((wq, bq, 1.0), (wk, bk, 1.0), (wv, bv, g))
        ):
            w = np.asarray(w, np.float32)
            bias = np.asarray(bias, np.float32)
            for hl in range(HPC):
                rows = slice(8 * (2 * hp + hl), 8 * (2 * hp + hl) + 8)
                cols = slice(sec * 16 + hl * 8, sec * 16 + hl * 8 + 8)
                wcat[:C, cols] = w[rows, :].T * s
                wcat[C, cols] = bias[rows] * s
        in_maps.append(
            {
                "xb": np.ascontiguousarray(x[b].reshape(C, N)).astype(NPBF16),
                "wcat": wcat.astype(NPBF16),
            }
        )
    return in_maps


def assemble_out(results, x):
    x = np.asarray(x, np.float32)
    out = np.empty((B, C, N), np.float32)
    for core in range(NCORES):
        b, hp = divmod(core, 4)
        oc = np.asarray(results[core]["out"], dtype=np.float32)  # [16, 4096]
        for hl in range(HPC):
            h = 2 * hp + hl
            # attn output permute(0,1,3,2).view(B,C,H,W): channel = 8h+dd,
            # spatial = n -- the device's [dd, n] tiles ARE the out rows.
            out[b, 8 * h : 8 * h + 8, :] = oc[hl * HD : (hl + 1) * HD, :]
    return out.reshape(B, C, 64, 64) + x


def kernel(x, wq, bq, wk, bk, wv, bv, gamma):
    nc = _get_nc()
    in_maps = make_in_maps(x, wq, bq, wk, bk, wv, bv, gamma)
    res = run_bass_kernel_spmd(nc, in_maps, list(range(NCORES))).results
    return assemble_out(res, x)


if __name__ == "__main__":
    rng = np.random.default_rng(0)
    x = rng.standard_normal((B, C, 64, 64), dtype=np.float32)
    wq, wk, wv = (
        rng.standard_normal((C, C), dtype=np.float32) / 8.0 for _ in range(3)
    )
    bq, bk, bv = (
        rng.standard_normal((C,), dtype=np.float32) * 0.01 for _ in range(3)
    )
    gamma = rng.random((1,), dtype=np.float32)
    out = kernel(x, wq, bq, wk, bk, wv, bv, gamma)
    print(out.shape, out.dtype)


# revision 13
# speedup vs baseline: 1.0741x; 1.0741x over previous
"""Multi-head attention (B=2, C=64, H=W=64, nh=8) on 8 TRN2 NeuronCores.

Sharding: core = (batch b, head-pair hp): b = core//4, heads {2hp, 2hp+1}
with hp = core%4.  Attention is independent per (batch, head), so 16 units
spread over 8 cores with zero communication; conv1x1 weights are sliced
per-core (each core only computes the 48 q/k/v output channels its two
heads need).

Per-core device pipeline (scores never hit HBM):
  x[b] bf16 [64,4096] -> conv1x1 on PE (bias via ones row) -> A [48,4096]
  bf16 (rows: q 2x8 | k 2x8 | v 2x8, torch-.view-scrambled layout
  A[row, 8t+dd]).
  - DVE deinterleave: Bqk [32, (dd,t)] bf16 (strided in-partition copy;
    kills the 2-byte-granular DMA gathers that dominated the first
    kernel -- Q^T/K^T gathers below move 1KB contiguous runs).
  - DRAM bounce: Bqk -> scrB, A_v -> scrA; per head gather
    qt/kt [8,4096] bf16 and vc [128,32,33] bf16 (16-byte runs; col 32 =
    ones for the softmax denominator).  All gathers for both heads are
    issued before the attention loops so head 1's DMA overlaps head 0's
    compute.
  - S^T tiles [128m, 512n] = K Q^T on PE (bf16), exp on ACT with
    1/sqrt(8) folded into the activation scale (scores are O(1) by
    construction; no max subtraction), PV + denominator via augmented
    V|ones bf16 matmul, normalize with PE ones-broadcast + DVE
    reciprocal.  The PV accumulator psum is copied to SBUF immediately so
    the slow reciprocal sits off the PV-accumulate critical path.
  Device returns gamma*attn^T per head ([dd, n] layout, bf16); the host
  un-views to [B,C,H,W] and adds the residual x in f32 (gamma is folded
  into the V weights).

Measured on hw: PE issues one 512-col matmul per ~427ns regardless of
dtype (fp8 DoubleRow only deepens contraction, useless at hd=8), so the
1024 S+PV matmuls put the per-core floor at ~440us; ACT exp is ~264us.
"""

import os

os.environ.setdefault("NEURON_RT_RESET_CORES", "1")  # recover wedged cores

import numpy as np
import ml_dtypes

import concourse.bacc as bacc
import concourse.bass as bass
import concourse.tile as tile
from concourse import mybir
from concourse.bass_utils import run_bass_kernel_spmd

F32 = mybir.dt.float32
F32R = mybir.dt.float32r
BF16 = mybir.dt.bfloat16

B = 2
C = 64
N = 4096          # H*W
NH = 8
HD = 8            # head dim
NCORES = 8
HPC = 2           # heads per core
NBLK = N // 512   # 8 n-blocks of 512 query positions
MCHUNK = N // 128  # 32 m-chunks of 128 key positions
EXPW = 1536       # elements exp'd per ACT instruction (psum banks = EXPW/512)
SCALE = 1.0 / np.sqrt(float(HD))
NPBF16 = ml_dtypes.bfloat16
NROWS = 3 * HPC * HD  # 48 conv output channels per core


def _chunk_groups():
    """Partition the 32 m-chunks into groups of <= EXPW//512 for one exp each."""
    per = EXPW // 512
    groups, k = [], 0
    while k < MCHUNK:
        n = min(per, MCHUNK - k)
        groups.append(list(range(k, k + n)))
        k += n
    return groups


def _emit(tc, xb_d, wcat_d, out_d, scrA, scrB):
    nc = tc.nc

    with (
        tc.tile_pool(name="persist", bufs=1) as per,
        tc.tile_pool(name="ptp", bufs=3) as ptp,
        tc.tile_pool(name="epl", bufs=4) as epl,
        tc.tile_pool(name="hdp", bufs=2) as hdp,
        tc.tile_pool(name="stp", bufs=2, space="PSUM") as stp,
        tc.tile_pool(name="accp", bufs=1, space="PSUM") as accp,
        tc.tile_pool(name="brgp", bufs=1, space="PSUM") as brgp,
    ):
        # ---- persistent: fp32r ones row for the denominator broadcast ----
        ones8r = per.tile([1, HD], F32R)
        o8f = per.tile([1, HD], F32)
        nc.vector.memset(o8f, 1.0)
        nc.vector.tensor_copy(ones8r, o8f)  # rounds to fp32r
        # Two PV accumulators share ONE psum bank (partition offsets 0/64,
        # both legal matmul col tile positions), freeing a bank for `brg`:
        # a bridge S-tile for chunk 0 of each block, so the next block's
        # S-matmuls never wait on the previous block's last exp.
        accT = accp.tile([97, 512], F32)
        brg = brgp.tile([128, 512], F32)

        # ---- conv1x1: [48,4096] = wcat.T @ [65,4096] ----
        with tc.tile_pool(name="convin", bufs=1) as cin:
            xa = cin.tile([C + 1, N], BF16)  # x[b] + ones row (bias)
            nc.vector.memset(xa[C : C + 1, :], 1.0)
            nc.sync.dma_start(out=xa[0:C, :], in_=xb_d[:])
            wc = cin.tile([C + 1, NROWS], BF16)
            nc.sync.dma_start(out=wc[:], in_=wcat_d[:])

            av = cin.tile([NROWS, N], BF16, name="av")  # conv out
            for j in range(NBLK):
                ps = stp.tile([128, EXPW], F32, tag="st")
                nc.tensor.matmul(
                    ps[0:NROWS, 0:512],
                    lhsT=wc[:],
                    rhs=xa[:, j * 512 : (j + 1) * 512],
                    start=True,
                    stop=True,
                )
                nc.vector.tensor_copy(
                    av[:, j * 512 : (j + 1) * 512], ps[0:NROWS, 0:512]
                )
            # V rows (natural layout) bounce to DRAM for the 16B-run gather
            nc.sync.dma_start(out=scrA[:], in_=av[32:48, :])
            # Q,K rows: deinterleave head-dim on DVE, bounce for 1KB gathers
            bqk = cin.tile([32, N], BF16, name="bqk")
            nc.vector.tensor_copy(
                bqk[:].rearrange("p (d t) -> p d t", d=HD),
                av[0:32, :].rearrange("p (t d) -> p d t", d=HD),
            )
            nc.sync.dma_start(out=scrB[:], in_=bqk[:])

        # ---- gathers for BOTH heads up front (overlap head 0 compute) ----
        qt, kt, vc = {}, {}, {}
        for h in range(HPC):
            qt[h] = hdp.tile([HD, N], BF16, name=f"qt{h}", tag="qt")
            kt[h] = hdp.tile([HD, N], BF16, name=f"kt{h}", tag="kt")
            vc[h] = hdp.tile([128, MCHUNK, 33], BF16, name=f"vc{h}", tag="vc")

            # qt[dd, 512r+t] = scrB[8h+r, dd*512 + t]  (1KB runs)
            nc.sync.dma_start(
                out=qt[h][:].rearrange("d (r t) -> d r t", r=HD),
                in_=scrB[h * HD : (h + 1) * HD, :].rearrange(
                    "r (d t) -> d r t", d=HD
                ),
            )
            nc.sync.dma_start(
                out=kt[h][:].rearrange("d (r t) -> d r t", r=HD),
                in_=scrB[16 + h * HD : 16 + (h + 1) * HD, :].rearrange(
                    "r (d t) -> d r t", d=HD
                ),
            )
            # V chunked [i, chunk, d]: m = 128*chunk + i, chunk = 4r+tb;
            # split into row-pair DMAs so the gather spreads across queues
            # and PV's first chunks are ready sooner.
            for q in range(4):
                rows = slice(h * HD + 2 * q, h * HD + 2 * q + 2)
                nc.sync.dma_start(
                    out=vc[h][:, 8 * q : 8 * q + 8, 0:HD],
                    in_=scrA[rows, :].rearrange(
                        "r (tb i d) -> i (r tb) d", tb=4, i=128, d=HD
                    ),
                )
            nc.vector.memset(vc[h][:, :, HD:32], 0.0)
            nc.vector.memset(vc[h][:, :, 32:33], 1.0)

        # ---- attention per (head, n-block) ----
        # Two-level software pipeline on the in-order PE queue:
        #  * 2-group lookahead: groups g+1, g+2's S-matmuls are emitted
        #    before group g's PV-matmuls so PE never waits on ACT's exp
        #    (or its cross-engine semaphore latency).
        #  * deferred epilogue: block j's denominator-broadcast matmul is
        #    emitted only after block j+1's matmuls, so PE doesn't sit
        #    behind the DVE denominator copy it depends on.
        # groups: chunk 0 rides the bridge tile; chunks 1..31 in 3s (last 1)
        per_g = EXPW // 512
        groups = [[0]]
        k = 1
        while k < MCHUNK:
            n = min(per_g, MCHUNK - k)
            groups.append(list(range(k, k + n)))
            k += n
        pending_epi = None
        blk_idx = 0
        for h in range(HPC):
            for j in range(NBLK):
                qblk = qt[h][:, j * 512 : (j + 1) * 512]
                acc = accT[0:33, :] if blk_idx % 2 == 0 else accT[64:97, :]
                blk_idx += 1

                def emit_s(gi, h=h, qblk=qblk):
                    grp = groups[gi]
                    st = brg if gi == 0 else stp.tile([128, EXPW], F32, tag="st")
                    for u, k in enumerate(grp):
                        nc.tensor.matmul(
                            st[:, u * 512 : (u + 1) * 512],
                            lhsT=kt[h][:, k * 128 : (k + 1) * 128],
                            rhs=qblk,
                            start=True,
                            stop=True,
                        )
                    return st

                sts = [emit_s(0), emit_s(1)]
                for gi, grp in enumerate(groups):
                    w = len(grp) * 512
                    st_cur = sts.pop(0)
                    pt = ptp.tile([128, EXPW], BF16)
                    nc.scalar.activation(
                        pt[:, 0:w], st_cur[:, 0:w],
                        mybir.ActivationFunctionType.Exp, scale=SCALE
                    )
                    if gi >= 1 and gi + 1 < len(groups):
                        sts.append(emit_s(gi + 1))
                    if gi == 1 and pending_epi is not None:
                        pending_epi()
                        pending_epi = None
                    for u, k in enumerate(grp):
                        nc.tensor.matmul(
                            acc[:, :],
                            lhsT=vc[h][:, k, :],
                            rhs=pt[:, u * 512 : (u + 1) * 512],
                            start=(k == 0),
                            stop=(k == MCHUNK - 1),
                        )

                # ---- epilogue: out = pv / denom (gamma pre-folded in V) ----
                # Copy psum acc to SBUF right away (frees acc for block j+2's
                # PV); the broadcast matmul + reciprocal are DEFERRED into
                # block j+1's emission so they never stall the PE queue.
                pv = epl.tile([HD, 512], F32, tag="pv")
                nc.vector.tensor_copy(pv, acc[0:HD, :])
                sb = epl.tile([1, 512], F32R, tag="sb")
                nc.vector.tensor_copy(sb, acc[32:33, :])  # denom -> fp32r

                def epilogue(h=h, j=j, pv=pv, sb=sb):
                    rbt = stp.tile([128, EXPW], F32, name="rbt", tag="st")
                    rb = rbt[0:HD, 0:512]
                    nc.tensor.matmul(
                        rb, lhsT=ones8r, rhs=sb, start=True, stop=True
                    )
                    rbs = epl.tile([HD, 512], F32, tag="rbs")
                    nc.vector.reciprocal(rbs, rb)
                    fin = epl.tile([HD, 512], BF16, tag="fin")
                    nc.vector.tensor_mul(fin, pv, rbs)
                    nc.gpsimd.dma_start(
                        out=out_d[
                            h * HD : (h + 1) * HD, j * 512 : (j + 1) * 512
                        ],
                        in_=fin,
                    )

                pending_epi = epilogue
        pending_epi()


def build_bass():
    nc = bacc.Bacc("TRN2", target_bir_lowering=False, debug=False, num_devices=NCORES)
    xb_d = nc.dram_tensor("xb", [C, N], BF16, kind="ExternalInput").ap()
    wcat_d = nc.dram_tensor("wcat", [C + 1, NROWS], BF16, kind="ExternalInput").ap()
    out_d = nc.dram_tensor("out", [HPC * HD, N], BF16, kind="ExternalOutput").ap()
    scrA = nc.dram_tensor("scrA", [16, N], BF16).ap()
    scrB = nc.dram_tensor("scrB", [32, N], BF16).ap()

    with tile.TileContext(nc) as tc:
        _emit(tc, xb_d, wcat_d, out_d, scrA, scrB)
    nc.finalize()
    return nc


_NC = None


def _get_nc():
    global _NC
    if _NC is None:
        _NC = build_bass()
    return _NC


def make_in_maps(x, wq, bq, wk, bk, wv, bv, gamma):
    x = np.asarray(x, np.float32)
    g = float(np.asarray(gamma, np.float32).reshape(-1)[0])
    in_maps = []
    for core in range(NCORES):
        b, hp = divmod(core, 4)
        wcat = np.empty((C + 1, NROWS), np.float32)
        for sec, (w, bias, s) in enumerate(
            ((wq, bq, 1.0), (wk, bk, 1.0), (wv, bv, g))
        ):
            w = np.asarray(w, np.float32)
            bias = np.asarray(bias, np.float32)
            for hl in range(HPC):
                rows = slice(8 * (2 * hp + hl), 8 * (2 * hp + hl) + 8)
                cols = slice(sec * 16 + hl * 8, sec * 16 + hl * 8 + 8)
                wcat[:C, cols] = w[rows, :].T * s
                wcat[C, cols] = bias[rows] * s
        in_maps.append(
            {
                "xb": np.ascontiguousarray(x[b].reshape(C, N)).astype(NPBF16),
                "wcat": wcat.astype(NPBF16),
            }
        )
    return in_maps


def assemble_out(results, x):
    x = np.asarray(x, np.float32)
    out = np.empty((B, C, N), np.float32)
    for core in range(NCORES):
        b, hp = divmod(core, 4)
        oc = np.asarray(results[core]["out"], dtype=np.float32)  # [16, 4096]
        for hl in range(HPC):
            h = 2 * hp + hl
            # attn output permute(0,1,3,2).view(B,C,H,W): channel = 8h+dd,
            # spatial = n -- the device's [dd, n] tiles ARE the out rows.
            out[b, 8 * h : 8 * h + 8, :] = oc[hl * HD : (hl + 1) * HD, :]
    return out.reshape(B, C, 64, 64) + x


def kernel(x, wq, bq, wk, bk, wv, bv, gamma):
    nc = _get_nc()
    in_maps = make_in_maps(x, wq, bq, wk, bk, wv, bv, gamma)
    res = run_bass_kernel_spmd(nc, in_maps, list(range(NCORES))).results
    return assemble_out(res, x)


if __name__ == "__main__":
    rng = np.random.default_rng(0)
    x = rng.standard_normal((B, C, 64, 64), dtype=np.float32)
    wq, wk, wv = (
        rng.standard_normal((C, C), dtype=np.float32) / 8.0 for _ in range(3)
    )
    bq, bk, bv = (
        rng.standard_normal((C,), dtype=np.float32) * 0.01 for _ in range(3)
    )
    gamma = rng.random((1,), dtype=np.float32)
    out = kernel(x, wq, bq, wk, bk, wv, bv, gamma)
    print(out.shape, out.dtype)
